# revision 36
# baseline (speedup 1.0000x reference)
"""Bass/Trainium2 kernel for nn_Decoder (2-layer bidir-style LSTM decoder
with general attention + fc), distributed over 8 NeuronCores.

v2 architecture (SPMD, one uniform program; per-core behavior differs only
in input DATA):
  - 4 LSTM cells (L0f, L0b, L1f, L1b) -> cores 0..3; cores 4..7 mirror the
    cells but own DIFFERENT post-phase batches. Scan is chunked, CH=32
    steps; L1 runs one chunk behind L0.
  - AllGather is split into two half-chunk collectives per iteration
    (issued after step 15 / step 31) over 4-rank groups, so transfer time
    hides under the recurrence.
  - ih precompute (W_emb @ emb + W_in @ xh + b) runs as two half-chunk
    passes interleaved into the per-step nonlinearity gaps of the
    surrounding iterations (keeps PE warm, off critical path).
  - Per step: gates = Whh @ h (4 k-chunks) + identity-matmul injection of
    the precomputed ih column (start=False accumulate), so the sigmoid
    reads PSUM directly and the old DVE add + sync gap disappears.
  - Post phase (attention + fc) is batch-split: core c handles batches
    2c, 2c+1 only, in transposed-score form:
      scoreT[s,t] = epT-tiles @ decT, exp(score + maskT) with the ragged
      mask folded into the ACT bias (no max subtraction; fp32 range is
      sufficient), sumexp via ones-matmul partition reduction, attention
      normalized through an outer-product broadcast matmul, ctxT and fc
      as dense matmuls with bias injected by rank-1 matmuls.
    Post work for chunk-group g is interleaved into iteration 4g+6's step
    gaps (g=3 is a short tail). Host stitches the 8 per-core outputs.

Numerics: bf16 weights/activations, fp32 PSUM + fp32 cell state c.
"""

import os
import sys

sys.path.insert(0, "/opt/trn_rl_repo")

import numpy as np
import ml_dtypes

import concourse.bass as bass
import concourse.mybir as mybir
import concourse.tile as tile
from concourse import bacc
from concourse.bass_utils import run_bass_kernel_spmd
from concourse.masks import make_identity

# ---- problem constants (hardcoded per contract) ----
L = 2
H = 512
E = 512
B = 16
T = 512
S = 512
VOCAB = 1001
OUT = 1000

N_CORES = 8
GS = 8                        # AllGather replica-group size (8-rank world group)
CH = 32                       # timesteps per chunk
NCH = T // CH                 # 16 chunks
ITERS = NCH + 1               # L1 lags one chunk
COLS = CH * B                 # 512 columns per chunk (s-major, b-minor)
HCOL = COLS // 2              # 256 columns per half chunk
HC = H // 128                 # 4 H-chunks
MC = (4 * H) // 128           # 16 gate M-chunks
SC = S // 128                 # 4 S-chunks
TG = 4                        # t-groups for post (128 steps each)
NB = 2                        # batches per core in post phase
NH = OUT // 2                 # fc output half width (psum bank limit)
BF = mybir.dt.bfloat16
F32 = mybir.dt.float32


# gate permutation: torch order i,f,g,o -> i,f,o,g  (rows of the 4H dim)
def _gate_perm():
    idx = np.arange(4 * H)
    return np.concatenate([idx[0:H], idx[H:2 * H], idx[3 * H:4 * H], idx[2 * H:3 * H]])


def _bf(x):
    return np.ascontiguousarray(np.asarray(x, dtype=np.float32)).astype(ml_dtypes.bfloat16)


def _f32(x):
    return np.ascontiguousarray(np.asarray(x, dtype=np.float32))


def build_nc(nch=NCH):
    assert nch == NCH, "v2 kernel supports full problem size only"
    iters = ITERS
    nc = bacc.Bacc("TRN2", target_bir_lowering=False, debug=False, num_devices=N_CORES)

    # ---- DRAM inputs ----
    w_emb = nc.dram_tensor("w_emb", [E, 4 * H], BF, kind="ExternalInput")
    w_in = nc.dram_tensor("w_in", [2 * H, 4 * H], BF, kind="ExternalInput")
    w_hh = nc.dram_tensor("w_hh", [H, 4 * H], BF, kind="ExternalInput")
    biasp = nc.dram_tensor("biasp", [128, MC], F32, kind="ExternalInput")
    h_init = nc.dram_tensor("h_init", [128, HC, B], BF, kind="ExternalInput")
    c_init = nc.dram_tensor("c_init", [128, HC, B], F32, kind="ExternalInput")
    alpha = nc.dram_tensor("alpha", [128, 1], F32, kind="ExternalInput")
    beta = nc.dram_tensor("beta", [128, 1], F32, kind="ExternalInput")
    emb_stream = nc.dram_tensor("emb_stream", [iters, E, COLS], BF, kind="ExternalInput")
    enc_lhsT = nc.dram_tensor("enc_lhsT", [NB, S, H], BF, kind="ExternalInput")
    encT_rhs = nc.dram_tensor("encT_rhs", [NB, H, S], BF, kind="ExternalInput")
    w_attT = nc.dram_tensor("w_attT", [E, H], BF, kind="ExternalInput")
    b_att_in = nc.dram_tensor("b_att_in", [128, HC], F32, kind="ExternalInput")
    maskT_in = nc.dram_tensor("maskT_in", [NB, 128, SC], F32, kind="ExternalInput")
    valid_in = nc.dram_tensor("valid_in", [NB, 128, TG], F32, kind="ExternalInput")
    w_fcT = nc.dram_tensor("w_fcT", [2 * H, OUT], BF, kind="ExternalInput")
    b_fc_row = nc.dram_tensor("b_fc_row", [1, OUT], BF, kind="ExternalInput")
    out_d = nc.dram_tensor("out", [NB, T, OUT], F32, kind="ExternalOutput")

    # ---- DRAM internals (collective bounce buffers) ----
    ag_out1 = nc.dram_tensor("ag_out1", [GS, H, HCOL], BF, addr_space="Shared")
    ag_out2 = nc.dram_tensor("ag_out2", [GS, H, HCOL], BF, addr_space="Shared")
    gamma = nc.dram_tensor("gamma", [128, 1], F32, kind="ExternalInput")

    if GS == 4:
        groups = [[0, 1, 2, 3], [4, 5, 6, 7]]
    else:
        groups = [list(range(N_CORES))]

    with tile.TileContext(nc) as tc:
        with (
            tc.tile_pool(name="wpool", bufs=1) as wpool,
            tc.tile_pool(name="spool", bufs=2) as spool,
            tc.tile_pool(name="steppool", bufs=3) as steppool,
            tc.tile_pool(name="pg", bufs=2, space="PSUM") as pg,
            tc.tile_pool(name="pih", bufs=3, space="PSUM") as pih,
            tc.tile_pool(name="ppost", bufs=2, space="PSUM") as ppost,
            tc.tile_pool(name="dpool", bufs=2, space="DRAM") as dpool,
        ):
            # ---- persistent SBUF ----
            wemb_sb = wpool.tile([128, HC, 4 * H], BF, tag="wemb")
            nc.sync.dma_start(wemb_sb[:], w_emb.rearrange("(k p) m -> p k m", p=128))
            win_sb = wpool.tile([128, 2 * HC, 4 * H], BF, tag="win")
            nc.sync.dma_start(win_sb[:], w_in.rearrange("(k p) m -> p k m", p=128))
            whh_sb = wpool.tile([128, HC, 4 * H], BF, tag="whh")
            nc.sync.dma_start(whh_sb[:], w_hh.rearrange("(k p) m -> p k m", p=128))
            biasp_sb = wpool.tile([128, MC], F32, tag="biasp")
            nc.sync.dma_start(biasp_sb[:], biasp[:])
            hinit_sb = wpool.tile([128, HC, B], BF, tag="hinit")
            nc.sync.dma_start(hinit_sb[:], h_init[:])
            cinit_sb = wpool.tile([128, HC, B], F32, tag="cinit")
            nc.sync.dma_start(cinit_sb[:], c_init[:])
            alpha_sb = wpool.tile([128, 1], F32, tag="alpha")
            nc.sync.dma_start(alpha_sb[:], alpha[:])
            beta_sb = wpool.tile([128, 1], F32, tag="beta")
            nc.sync.dma_start(beta_sb[:], beta[:])
            gamma_sb = wpool.tile([128, 1], F32, tag="gamma")
            nc.sync.dma_start(gamma_sb[:], gamma[:])

            identity = wpool.tile([128, 128], BF, tag="ident")
            make_identity(nc, identity[:])
            ones_r = wpool.tile([1, 128], BF, tag="ones_r")
            nc.vector.memset(ones_r[:], 1.0)
            ones_p = wpool.tile([128, 1], BF, tag="ones_p")
            nc.vector.memset(ones_p[:], 1.0)

            wattT_sb = wpool.tile([128, HC, H], BF, tag="wattT")
            nc.sync.dma_start(wattT_sb[:], w_attT.rearrange("(k p) m -> p k m", p=128))
            batt_sb = wpool.tile([128, HC], F32, tag="batt")
            nc.sync.dma_start(batt_sb[:], b_att_in[:])
            wfc_sb = wpool.tile([128, 2 * HC, OUT], BF, tag="wfc")
            nc.sync.dma_start(wfc_sb[:], w_fcT.rearrange("(k p) m -> p k m", p=128))
            bfc_sb = wpool.tile([1, OUT], BF, tag="bfc")
            nc.sync.dma_start(bfc_sb[:], b_fc_row[:])
            encT_sb = wpool.tile([128, NB, HC, S], BF, tag="encT")
            nc.sync.dma_start(
                encT_sb[:], encT_rhs.rearrange("b (k p) s -> p b k s", p=128)
            )
            enc_sb = wpool.tile([128, NB, SC, H], BF, tag="enc")
            nc.sync.dma_start(
                enc_sb[:], enc_lhsT.rearrange("b (k p) h -> p b k h", p=128)
            )
            maskT_sb = wpool.tile([128, NB, SC], F32, tag="maskT")
            nc.sync.dma_start(maskT_sb[:], maskT_in.rearrange("b p k -> p b k"))
            valid_sb = wpool.tile([128, NB, TG], F32, tag="valid")
            nc.sync.dma_start(valid_sb[:], valid_in.rearrange("b p k -> p b k"))

            # dec accumulator: [128, group, hk, t_in_group, batch]
            dec_all = wpool.tile([128, TG, HC, 128, NB], BF, tag="dec_all")
            # enc_proj^T per post batch: [128, b, hk, S]
            epT_sb = wpool.tile([128, NB, HC, S], BF, tag="epT")

            # ---------- prologue: ih half0 for iteration 0 (xh == 0) ----------
            emb_t = spool.tile([128, HC, COLS], BF, tag="emb_t")
            nc.sync.dma_start(emb_t[:], emb_stream[0].rearrange("(k p) c -> p k c", p=128))
            ih_cur = spool.tile([128, MC, COLS], BF, tag="ih")

            def ih0_mchunk(m, half, emb_tile, ih_tile):
                c0, c1 = half * HCOL, (half + 1) * HCOL

                def run():
                    ps = pih.tile([128, HCOL], F32, tag="ihps")
                    for kk in range(HC):
                        nc.tensor.matmul(
                            ps[:], wemb_sb[:, kk, m * 128:(m + 1) * 128],
                            emb_tile[:, kk, c0:c1], start=(kk == 0), stop=(kk == HC - 1),
                        )
                    nc.gpsimd.tensor_scalar_add(
                        ih_tile[:, m, c0:c1], ps[:], biasp_sb[:, m:m + 1]
                    )
                return run

            for m in range(MC):
                ih0_mchunk(m, 0, emb_t, ih_cur)()

            def epT_thunk(b, hk):
                def run():
                    pp = ppost.tile([128, S], F32, tag="pp")
                    for e in range(HC):
                        nc.tensor.matmul(
                            pp[:], wattT_sb[:, e, hk * 128:(hk + 1) * 128],
                            encT_sb[:, b, e, :], start=(e == 0), stop=(e == HC - 1),
                        )
                    nc.gpsimd.tensor_scalar_add(
                        epT_sb[:, b, hk, :], pp[:], batt_sb[:, hk:hk + 1]
                    )
                return run

            # =====================================================
            # helper closures
            # =====================================================
            def make_ih_half(k_iter, half, emb_tile, ih_tile, ag_src):
                """Thunks for ih of iteration k_iter, half columns; xh from ag_src."""
                thunks = []
                xh_box = {}

                def load_xh():
                    xh = spool.tile([128, 2 * HC, HCOL], BF, tag=f"xh{half}")
                    if ag_src is None:
                        nc.vector.memset(xh[:], 0.0)
                    else:
                        nc.sync.dma_start(
                            xh[:], ag_src[0:2].rearrange("s (k p) c -> p (s k) c", p=128)
                        )
                    xh_box["t"] = xh

                thunks.append(load_xh)
                c0, c1 = half * HCOL, (half + 1) * HCOL

                def mchunk(m):
                    def run():
                        xh = xh_box["t"]
                        ps = pih.tile([128, HCOL], F32, tag="ihps")
                        for kk in range(HC):
                            nc.tensor.matmul(
                                ps[:], wemb_sb[:, kk, m * 128:(m + 1) * 128],
                                emb_tile[:, kk, c0:c1], start=(kk == 0), stop=False,
                            )
                        for kk in range(2 * HC):
                            nc.tensor.matmul(
                                ps[:], win_sb[:, kk, m * 128:(m + 1) * 128],
                                xh[:, kk, :], start=False, stop=(kk == 2 * HC - 1),
                            )
                        nc.gpsimd.tensor_scalar_add(
                            ih_tile[:, m, c0:c1], ps[:], biasp_sb[:, m:m + 1]
                        )
                    return run

                for m in range(MC):
                    thunks.append(mchunk(m))
                return thunks

            def make_dec_half(chunk, half, accum_src):
                """Thunks: ReduceScatter the masked L1 h half-chunk so each core
                receives dec = h1f + h1b for ITS post batches. Cores with
                gamma=0 contribute zeros; cores 2,3 (L1f/L1b) contribute their
                h sliced per receiver batch; add happens in the collective."""
                g, j = chunk // 4, chunk % 4
                pos = j * 32 + half * 16
                c0 = (1 + half * 16) * B
                box = {}

                def send():
                    # b-major masked copy so per-receiver slices are contiguous
                    hm = spool.tile([128, HC, B, 16], BF, tag="hm", bufs=1)
                    nc.gpsimd.tensor_scalar_mul(
                        hm[:],
                        accum_src[:, :, c0:c0 + HCOL].rearrange(
                            "p k (s b) -> p k b s", b=B
                        ),
                        gamma_sb[:, 0:1],
                    )
                    rs_in = dpool.tile([GS, H, NB * 16], BF, tag="rs_in")
                    for j2 in range(GS):
                        nc.sync.dma_start(
                            rs_in[j2].rearrange("(k p) (b s) -> p k b s", p=128, b=NB),
                            hm[:, :, NB * j2:NB * (j2 + 1), :],
                        )
                    rs_out = dpool.tile([H, NB * 16], BF, tag="rs_out")
                    nc.gpsimd.collective_compute(
                        "ReduceScatter", mybir.AluOpType.add, replica_groups=groups,
                        ins=[rs_in[:].opt()], outs=[rs_out[:].opt()],
                    )
                    box["t"] = rs_out

                def recv():
                    stg = spool.tile([128, HC, NB, 16], BF, tag="decstg", bufs=2)
                    nc.sync.dma_start(
                        stg[:], box["t"].rearrange("(k p) (b s) -> p k b s", p=128, b=NB)
                    )
                    nc.gpsimd.tensor_copy(
                        dec_all[:, g, :, pos:pos + 16, :],
                        stg.rearrange("p k b s -> p k s b"),
                    )

                return [send, recv]

            def make_post(g):
                """Thunks for post phase of t-group g (128 timesteps), NB batches."""
                thunks = []
                for b in range(NB):
                    state = {}

                    def score_sk(b, sk, state=state):
                        def run():
                            sp = ppost.tile([128, 128], F32, tag="pp")
                            for hk in range(HC):
                                nc.tensor.matmul(
                                    sp[:], epT_sb[:, b, hk, sk * 128:(sk + 1) * 128],
                                    dec_all[:, g, hk, :, b],
                                    start=(hk == 0), stop=(hk == HC - 1),
                                )
                            if sk == 0:
                                expT = spool.tile([128, SC, 128], BF, tag="expT", bufs=2)
                                state["expT"] = expT
                            nc.scalar.activation(
                                state["expT"][:, sk, :], sp[:],
                                mybir.ActivationFunctionType.Exp,
                                bias=maskT_sb[:, b, sk:sk + 1],
                            )
                        return run

                    def norm(b, state=state):
                        def run():
                            expT = state["expT"]
                            se = ppost.tile([1, 128], F32, tag="pp")
                            for sk in range(SC):
                                nc.tensor.matmul(
                                    se[:], ones_p[:, 0:1], expT[:, sk, :],
                                    start=(sk == 0), stop=(sk == SC - 1),
                                )
                            recip = steppool.tile([1, 128], BF, tag="recip")
                            with nc.allow_low_precision(reason="bf16 attention normalizer"):
                                nc.vector.reciprocal(recip[:], se[:])
                            bc = ppost.tile([128, 128], F32, tag="pp")
                            nc.tensor.matmul(
                                bc[:], ones_r[:, :], recip[:], start=True, stop=True,
                            )
                            att = spool.tile([128, SC, 128], BF, tag="att", bufs=2)
                            for sk in range(SC):
                                nc.vector.tensor_mul(att[:, sk, :], expT[:, sk, :], bc[:])
                            state["att"] = att
                        return run

                    def ctx_hk(b, hk, state=state):
                        def run():
                            att = state["att"]
                            cp = ppost.tile([128, 128], F32, tag="pp")
                            for sk in range(SC):
                                nc.tensor.matmul(
                                    cp[:], enc_sb[:, b, sk, hk * 128:(hk + 1) * 128],
                                    att[:, sk, :], start=(sk == 0), stop=(sk == SC - 1),
                                )
                            if hk == 0:
                                ctxT = spool.tile([128, HC, 128], BF, tag="ctxT", bufs=2)
                                state["ctxT"] = ctxT
                            nc.vector.tensor_copy(state["ctxT"][:, hk, :], cp[:])
                        return run

                    def fc_half(b, nh, state=state):
                        def run():
                            ctxT = state["ctxT"]
                            fp = ppost.tile([128, NH], F32, tag="pp")
                            for kk in range(2 * HC):
                                lhs = (dec_all[:, g, kk, :, b] if kk < HC
                                       else ctxT[:, kk - HC, :])
                                nc.tensor.matmul(
                                    fp[:], lhs, wfc_sb[:, kk, nh * NH:(nh + 1) * NH],
                                    start=(kk == 0), stop=False,
                                )
                            nc.tensor.matmul(
                                fp[:], ones_r[:, :], bfc_sb[:, nh * NH:(nh + 1) * NH],
                                start=False, stop=True,
                            )
                            osb = steppool.tile([128, NH], F32, tag="osb", bufs=2)
                            nc.vector.tensor_scalar_mul(osb[:], fp[:], valid_sb[:, b, g:g + 1])
                            nc.sync.dma_start(
                                out_d[b, g * 128:(g + 1) * 128, nh * NH:(nh + 1) * NH],
                                osb[:],
                            )
                        return run

                    for sk in range(SC):
                        thunks.append(score_sk(b, sk))
                    thunks.append(norm(b))
                    for hk in range(HC):
                        thunks.append(ctx_hk(b, hk))
                    for nh in range(2):
                        thunks.append(fc_half(b, nh))
                return thunks

            # =====================================================
            # main scan
            # =====================================================
            accum_prev = None
            c_cur = None
            emb_next = None
            ih_next = None
            # iter-0 gap work: ih0 half1 (emb-only) + enc_proj^T build
            pend_half1 = [ih0_mchunk(m, 1, emb_t, ih_cur) for m in range(MC)]
            pend_post = [epT_thunk(b, hk) for b in range(NB) for hk in range(HC)]

            for k in range(iters):
                # fresh per-iteration stream tiles
                if k > 0:
                    emb_t = emb_next
                    ih_cur = ih_next
                if k + 1 < iters:
                    emb_next = spool.tile([128, HC, COLS], BF, tag="emb_t")
                    nc.sync.dma_start(
                        emb_next[:], emb_stream[k + 1].rearrange("(k p) c -> p k c", p=128)
                    )
                    ih_next = spool.tile([128, MC, COLS], BF, tag="ih")

                # ---------- state carry / blend ----------
                accum = spool.tile([128, HC, (CH + 1) * B], BF, tag="accum")
                if k == 0:
                    nc.vector.tensor_copy(accum[:, :, 0:B], hinit_sb[:])
                    c_new0 = steppool.tile([128, HC, B], F32, tag="c")
                    nc.vector.tensor_copy(c_new0[:], cinit_sb[:])
                    c_cur = c_new0
                elif k == 1:
                    t1 = steppool.tile([128, HC, B], F32, tag="blend")
                    nc.vector.tensor_scalar_mul(t1[:], accum_prev[:, :, CH * B:], alpha_sb[:, 0:1])
                    t2 = steppool.tile([128, HC, B], F32, tag="blend")
                    nc.vector.tensor_scalar_mul(t2[:], hinit_sb[:], beta_sb[:, 0:1])
                    nc.vector.tensor_add(accum[:, :, 0:B], t1[:], t2[:])
                    t3 = steppool.tile([128, HC, B], F32, tag="blend")
                    nc.vector.tensor_scalar_mul(t3[:], c_cur[:], alpha_sb[:, 0:1])
                    t4 = steppool.tile([128, HC, B], F32, tag="blend")
                    nc.vector.tensor_scalar_mul(t4[:], cinit_sb[:], beta_sb[:, 0:1])
                    c_new1 = steppool.tile([128, HC, B], F32, tag="c")
                    nc.vector.tensor_add(c_new1[:], t3[:], t4[:])
                    c_cur = c_new1
                else:
                    nc.vector.tensor_copy(accum[:, :, 0:B], accum_prev[:, :, CH * B:])

                # ---------- filler schedule for this iteration ----------
                # fillers[s] = list of thunks issued right after step s's matmuls
                fillers = [[] for _ in range(CH)]
                for i, t in enumerate(pend_half1):
                    fillers[min(4 + i, 15)].append(t)
                for i, t in enumerate(pend_post):
                    fillers[min(8 + i, CH - 2)].append(t)
                pend_half0 = []
                if k + 1 < iters:
                    pend_half0 = make_ih_half(k + 1, 0, emb_next, ih_next, ag_out1)
                if k >= 1:
                    pend_half0 = make_dec_half(k - 1, 0, accum) + pend_half0
                for i, t in enumerate(pend_half0):
                    fillers[min(19 + i, CH - 1)].append(t)

                # ---------- CH recurrence steps ----------
                for s in range(CH):
                    if s == 16:
                        ag_in1 = dpool.tile([H, HCOL], BF, tag="ag_in1")
                        nc.sync.dma_start(
                            ag_in1.rearrange("(k p) c -> p k c", p=128),
                            accum[:, :, B:(1 + 16) * B],
                        )
                        nc.gpsimd.collective_compute(
                            "AllGather", mybir.AluOpType.bypass, replica_groups=groups,
                            ins=[ag_in1.opt()], outs=[ag_out1[:].opt()],
                        )

                    g_ps = pg.tile([128, MC, B], F32, tag="g")
                    # inject precomputed ih for this step into PSUM in one
                    # identity matmul (start=True clears), then accumulate the
                    # Whh contraction on top. Group shapes differ -> skip check.
                    nc.tensor.matmul(
                        g_ps[:], identity[:],
                        ih_cur[:, :, s * B:(s + 1) * B],
                        start=True, stop=False, skip_group_check=True,
                    )
                    for m in range(MC):
                        for kk in range(HC):
                            nc.tensor.matmul(
                                g_ps[:, m, :],
                                whh_sb[:, kk, m * 128:(m + 1) * 128],
                                accum[:, kk, s * B:(s + 1) * B],
                                start=False, stop=(kk == HC - 1),
                                skip_group_check=True,
                            )

                    for t in fillers[s]:
                        t()

                    sig = steppool.tile([128, 12, B], F32, tag="sig")
                    nc.scalar.activation(sig[:], g_ps[:, 0:12, :], mybir.ActivationFunctionType.Sigmoid)
                    tg = steppool.tile([128, HC, B], F32, tag="tg")
                    nc.scalar.activation(tg[:], g_ps[:, 12:16, :], mybir.ActivationFunctionType.Tanh)
                    m1 = steppool.tile([128, HC, B], F32, tag="m1")
                    nc.vector.tensor_mul(m1[:], sig[:, 4:8, :], c_cur[:])
                    m2 = steppool.tile([128, HC, B], F32, tag="m2")
                    nc.vector.tensor_mul(m2[:], sig[:, 0:4, :], tg[:])
                    c_new = steppool.tile([128, HC, B], F32, tag="c")
                    nc.vector.tensor_add(c_new[:], m1[:], m2[:])
                    tc_t = steppool.tile([128, HC, B], F32, tag="tc")
                    nc.scalar.activation(tc_t[:], c_new[:], mybir.ActivationFunctionType.Tanh)
                    nc.vector.tensor_mul(accum[:, :, (s + 1) * B:(s + 2) * B], sig[:, 8:12, :], tc_t[:])
                    c_cur = c_new

                # ---------- second-half exchange ----------
                ag_in2 = dpool.tile([H, HCOL], BF, tag="ag_in2")
                nc.sync.dma_start(
                    ag_in2.rearrange("(k p) c -> p k c", p=128),
                    accum[:, :, (1 + 16) * B:(1 + 32) * B],
                )
                nc.gpsimd.collective_compute(
                    "AllGather", mybir.AluOpType.bypass, replica_groups=groups,
                    ins=[ag_in2.opt()], outs=[ag_out2[:].opt()],
                )

                # ---------- queue work that depends on AG2_k ----------
                pend_half1 = []
                if k + 1 < iters:
                    pend_half1 = make_ih_half(k + 1, 1, emb_next, ih_next, ag_out2)
                if k >= 1:
                    pend_half1 = make_dec_half(k - 1, 1, accum) + pend_half1
                pend_post = []
                if k + 1 >= 6 and (k + 1 - 6) % 4 == 0 and (k + 1 - 6) // 4 < 3:
                    pend_post = make_post((k + 1 - 6) // 4)

                accum_prev = accum

            # ---------- tail: dec half1 of chunk 15 + post group 3 ----------
            for t in pend_half1:
                t()
            for t in make_post(3):
                t()

    nc.compile()
    return nc


# ---------------- host-side preparation ----------------

def _prep_inputs(inputs, nch=NCH):
    assert nch == NCH
    iters = ITERS
    perm = _gate_perm()

    trg = np.asarray(inputs["trg_inputs"]).astype(np.int64)
    trg_len = np.asarray(inputs["trg_len"]).astype(np.int64)
    enc = _f32(inputs["encoder_outputs"])
    h0 = _f32(inputs["h0"]).reshape(L, 2, B, H)
    c0 = _f32(inputs["c0"]).reshape(L, 2, B, H)
    embed = _f32(inputs["embed"])
    W_ih0 = _f32(inputs["W_ih0"])          # [2, 4H, E]
    W_ih1 = _f32(inputs["W_ih1"])[0]       # [2, 4H, 2H]
    W_hh = _f32(inputs["W_hh"])            # [L, 2, 4H, H]
    b_ih = _f32(inputs["b_ih"])            # [L, 2, 4H]
    b_hh = _f32(inputs["b_hh"])
    W_att = _f32(inputs["W_att"])          # [H, H]
    b_att = _f32(inputs["b_att"])          # [H]
    W_fc = _f32(inputs["W_fc"])            # [OUT, 2H]
    b_fc = _f32(inputs["b_fc"])            # [OUT]

    # embedding stream  [iters, E, COLS]; emb_stream[k,e,s*B+b] = X[b,32k+s,e]
    X = embed[trg[:, :T]]                             # [B, T, E]
    es = np.zeros((iters, E, COLS), np.float32)
    xt = X.transpose(2, 1, 0)                         # [E, T, B]
    es[:NCH] = (
        xt.reshape(E, NCH, CH, B).transpose(1, 0, 2, 3).reshape(NCH, E, COLS)
    )
    es = _bf(es)

    cells = [(0, 0), (0, 1), (1, 0), (1, 1)]          # (layer, dir)
    zeros_emb = _bf(np.zeros((E, 4 * H)))
    zeros_in = _bf(np.zeros((2 * H, 4 * H)))

    # masks in transposed layouts
    # maskT [B, 128, SC]: 0 where s < len else -1e30  (s = sk*128 + p)
    # -30 shift guards exp() against fp32 overflow (scores can reach ~90);
    # the shift cancels in the softmax normalization.
    s_idx = np.arange(S).reshape(SC, 128).T           # [128, SC]
    maskT = np.where(s_idx[None, :, :] < trg_len[:, None, None], -30.0, -1e30).astype(np.float32)
    # validT [B, 128, TG]: 1 where t < len else 0  (t = g*128 + p)
    t_idx = np.arange(T).reshape(TG, 128).T
    validT = (t_idx[None, :, :] < trg_len[:, None, None]).astype(np.float32)

    encT = enc.transpose(0, 2, 1)                     # [B, H, S]

    in_maps = []
    for c in range(N_CORES):
        cell = c % 4
        layer, d = cells[cell]
        if layer == 0:
            wemb = _bf(W_ih0[d][perm].T)              # [E, 4H]
            win = zeros_in
        else:
            wemb = zeros_emb
            win = _bf(W_ih1[d][perm].T)               # [2H, 4H]
        whh = _bf(W_hh[layer, d][perm].T)             # [H, 4H]
        bp = (b_ih[layer, d] + b_hh[layer, d])[perm]  # [4H]
        biasp = _f32(bp.reshape(MC, 128).T)           # [128, MC]
        hin = h0[layer, d]                            # [B, H]
        cin = c0[layer, d]
        h_init = _bf(hin.T.reshape(HC, 128, B).transpose(1, 0, 2))   # [128,HC,B]
        c_init = _f32(cin.T.reshape(HC, 128, B).transpose(1, 0, 2))
        a = 1.0 if layer == 0 else 0.0
        alpha = _f32(np.full((128, 1), a))
        beta = _f32(np.full((128, 1), 1.0 - a))
        gam = _f32(np.full((128, 1), 1.0 if c in (2, 3) else 0.0))

        bsl = [2 * c, 2 * c + 1]                      # post batches of this core

        m = dict(
            w_emb=wemb, w_in=win, w_hh=whh, biasp=biasp,
            h_init=h_init, c_init=c_init, alpha=alpha, beta=beta, gamma=gam,
            emb_stream=es,
            enc_lhsT=_bf(enc[bsl]),                   # [2, S, H]
            encT_rhs=_bf(encT[bsl]),                  # [2, H, S]
            w_attT=_bf(W_att.T),
            b_att_in=_f32(b_att.reshape(HC, 128).T),
            maskT_in=_f32(maskT[bsl]),                # [2,128,SC]
            valid_in=_f32(validT[bsl]),               # [2,128,TG]
            w_fcT=_bf(W_fc.T),                        # [2H, OUT]
            b_fc_row=_bf(b_fc[None, :]),
        )
        in_maps.append(m)
    return in_maps


_NC_CACHE = {}


def kernel(**inputs) -> np.ndarray:
    nch = int(os.environ.get("KERNEL_NCH", NCH))
    if nch not in _NC_CACHE:
        _NC_CACHE[nch] = build_nc(nch)
    nc = _NC_CACHE[nch]
    in_maps = _prep_inputs(inputs, nch)
    r = run_bass_kernel_spmd(nc, in_maps, list(range(N_CORES)))
    outs = [np.asarray(r.results[c]["out"], np.float32) for c in range(N_CORES)]
    return np.concatenate(outs, axis=0)


# revision 40
# speedup vs baseline: 1.1034x; 1.1034x over previous
"""Bass/Trainium2 kernel for nn_Decoder (2-layer bidir-style LSTM decoder
with general attention + fc), distributed over 8 NeuronCores.

v2 architecture (SPMD, one uniform program; per-core behavior differs only
in input DATA):
  - 4 LSTM cells (L0f, L0b, L1f, L1b) -> cores 0..3; cores 4..7 mirror the
    cells but own DIFFERENT post-phase batches. Scan is chunked, CH=32
    steps; L1 runs one chunk behind L0.
  - AllGather is split into two half-chunk collectives per iteration
    (issued after step 15 / step 31) over 4-rank groups, so transfer time
    hides under the recurrence.
  - ih precompute (W_emb @ emb + W_in @ xh + b) runs as two half-chunk
    passes interleaved into the per-step nonlinearity gaps of the
    surrounding iterations (keeps PE warm, off critical path).
  - Per step: gates = Whh @ h (4 k-chunks) + identity-matmul injection of
    the precomputed ih column (start=False accumulate), so the sigmoid
    reads PSUM directly and the old DVE add + sync gap disappears.
  - Post phase (attention + fc) is batch-split: core c handles batches
    2c, 2c+1 only, in transposed-score form:
      scoreT[s,t] = epT-tiles @ decT, exp(score + maskT) with the ragged
      mask folded into the ACT bias (no max subtraction; fp32 range is
      sufficient), sumexp via ones-matmul partition reduction, attention
      normalized through an outer-product broadcast matmul, ctxT and fc
      as dense matmuls with bias injected by rank-1 matmuls.
    Post work for chunk-group g is interleaved into iteration 4g+6's step
    gaps (g=3 is a short tail). Host stitches the 8 per-core outputs.

Numerics: bf16 weights/activations, fp32 PSUM + fp32 cell state c.
"""

import os
import sys

sys.path.insert(0, "/opt/trn_rl_repo")

import numpy as np
import ml_dtypes

import concourse.bass as bass
import concourse.mybir as mybir
import concourse.tile as tile
from concourse import bacc
from concourse.bass_utils import run_bass_kernel_spmd
from concourse.masks import make_identity

# ---- problem constants (hardcoded per contract) ----
L = 2
H = 512
E = 512
B = 16
T = 512
S = 512
VOCAB = 1001
OUT = 1000

N_CORES = 8
GS = 8                        # AllGather replica-group size (8-rank world group)
CH = 32                       # timesteps per chunk
NCH = T // CH                 # 16 chunks
ITERS = NCH + 1               # L1 lags one chunk
COLS = CH * B                 # 512 columns per chunk (s-major, b-minor)
HCOL = COLS // 2              # 256 columns per half chunk
HC = H // 128                 # 4 H-chunks
MC = (4 * H) // 128           # 16 gate M-chunks
SC = S // 128                 # 4 S-chunks
TG = 4                        # t-groups for post (128 steps each)
NB = 2                        # batches per core in post phase
NH = OUT // 2                 # fc output half width (psum bank limit)
BF = mybir.dt.bfloat16
F32 = mybir.dt.float32


# gate permutation: torch order i,f,g,o -> i,f,o,g  (rows of the 4H dim)
def _gate_perm():
    idx = np.arange(4 * H)
    return np.concatenate([idx[0:H], idx[H:2 * H], idx[3 * H:4 * H], idx[2 * H:3 * H]])


def _bf(x):
    return np.ascontiguousarray(np.asarray(x, dtype=np.float32)).astype(ml_dtypes.bfloat16)


def _f32(x):
    return np.ascontiguousarray(np.asarray(x, dtype=np.float32))


def build_nc(nch=NCH):
    assert nch == NCH, "v2 kernel supports full problem size only"
    iters = ITERS
    nc = bacc.Bacc("TRN2", target_bir_lowering=False, debug=False, num_devices=N_CORES)

    # ---- DRAM inputs ----
    w_emb = nc.dram_tensor("w_emb", [E, 4 * H], BF, kind="ExternalInput")
    w_in = nc.dram_tensor("w_in", [2 * H, 4 * H], BF, kind="ExternalInput")
    w_hh = nc.dram_tensor("w_hh", [H, 4 * H], BF, kind="ExternalInput")
    biasp = nc.dram_tensor("biasp", [128, MC], F32, kind="ExternalInput")
    h_init = nc.dram_tensor("h_init", [128, HC, B], BF, kind="ExternalInput")
    c_init = nc.dram_tensor("c_init", [128, HC, B], F32, kind="ExternalInput")
    alpha = nc.dram_tensor("alpha", [128, 1], F32, kind="ExternalInput")
    beta = nc.dram_tensor("beta", [128, 1], F32, kind="ExternalInput")
    emb_stream = nc.dram_tensor("emb_stream", [iters, E, COLS], BF, kind="ExternalInput")
    enc_lhsT = nc.dram_tensor("enc_lhsT", [NB, S, H], BF, kind="ExternalInput")
    encT_rhs = nc.dram_tensor("encT_rhs", [NB, H, S], BF, kind="ExternalInput")
    w_attT = nc.dram_tensor("w_attT", [E, H], BF, kind="ExternalInput")
    b_att_in = nc.dram_tensor("b_att_in", [128, HC], F32, kind="ExternalInput")
    maskT_in = nc.dram_tensor("maskT_in", [NB, 128, SC], F32, kind="ExternalInput")
    valid_in = nc.dram_tensor("valid_in", [NB, 128, TG], F32, kind="ExternalInput")
    w_fcT = nc.dram_tensor("w_fcT", [2 * H, OUT], BF, kind="ExternalInput")
    b_fc_row = nc.dram_tensor("b_fc_row", [1, OUT], BF, kind="ExternalInput")
    out_d = nc.dram_tensor("out", [NB, T, OUT], F32, kind="ExternalOutput")

    # ---- DRAM internals (collective bounce buffers) ----
    ag_out1 = nc.dram_tensor("ag_out1", [GS, H, HCOL], BF, addr_space="Shared")
    ag_out2 = nc.dram_tensor("ag_out2", [GS, H, HCOL], BF, addr_space="Shared")
    gamma = nc.dram_tensor("gamma", [128, 1], F32, kind="ExternalInput")

    if GS == 4:
        groups = [[0, 1, 2, 3], [4, 5, 6, 7]]
    else:
        groups = [list(range(N_CORES))]

    with tile.TileContext(nc) as tc:
        with (
            tc.tile_pool(name="wpool", bufs=1) as wpool,
            tc.tile_pool(name="spool", bufs=2) as spool,
            tc.tile_pool(name="steppool", bufs=3) as steppool,
            tc.tile_pool(name="pg", bufs=2, space="PSUM") as pg,
            tc.tile_pool(name="pih", bufs=3, space="PSUM") as pih,
            tc.tile_pool(name="ppost", bufs=2, space="PSUM") as ppost,
            tc.tile_pool(name="dpool", bufs=2, space="DRAM") as dpool,
        ):
            # ---- persistent SBUF ----
            wemb_sb = wpool.tile([128, HC, 4 * H], BF, tag="wemb")
            nc.sync.dma_start(wemb_sb[:], w_emb.rearrange("(k p) m -> p k m", p=128))
            win_sb = wpool.tile([128, 2 * HC, 4 * H], BF, tag="win")
            nc.sync.dma_start(win_sb[:], w_in.rearrange("(k p) m -> p k m", p=128))
            whh_sb = wpool.tile([128, HC, 4 * H], BF, tag="whh")
            nc.sync.dma_start(whh_sb[:], w_hh.rearrange("(k p) m -> p k m", p=128))
            biasp_sb = wpool.tile([128, MC], F32, tag="biasp")
            nc.sync.dma_start(biasp_sb[:], biasp[:])
            hinit_sb = wpool.tile([128, HC, B], BF, tag="hinit")
            nc.sync.dma_start(hinit_sb[:], h_init[:])
            cinit_sb = wpool.tile([128, HC, B], F32, tag="cinit")
            nc.sync.dma_start(cinit_sb[:], c_init[:])
            alpha_sb = wpool.tile([128, 1], F32, tag="alpha")
            nc.sync.dma_start(alpha_sb[:], alpha[:])
            beta_sb = wpool.tile([128, 1], F32, tag="beta")
            nc.sync.dma_start(beta_sb[:], beta[:])
            gamma_sb = wpool.tile([128, 1], F32, tag="gamma")
            nc.sync.dma_start(gamma_sb[:], gamma[:])

            identity = wpool.tile([128, 128], BF, tag="ident")
            make_identity(nc, identity[:])
            ones_r = wpool.tile([1, 128], BF, tag="ones_r")
            nc.vector.memset(ones_r[:], 1.0)
            ones_p = wpool.tile([128, 1], BF, tag="ones_p")
            nc.vector.memset(ones_p[:], 1.0)

            wattT_sb = wpool.tile([128, HC, H], BF, tag="wattT")
            nc.sync.dma_start(wattT_sb[:], w_attT.rearrange("(k p) m -> p k m", p=128))
            batt_sb = wpool.tile([128, HC], F32, tag="batt")
            nc.sync.dma_start(batt_sb[:], b_att_in[:])
            wfc_sb = wpool.tile([128, 2 * HC, OUT], BF, tag="wfc")
            nc.sync.dma_start(wfc_sb[:], w_fcT.rearrange("(k p) m -> p k m", p=128))
            bfc_sb = wpool.tile([1, OUT], BF, tag="bfc")
            nc.sync.dma_start(bfc_sb[:], b_fc_row[:])
            encT_sb = wpool.tile([128, NB, HC, S], BF, tag="encT")
            nc.sync.dma_start(
                encT_sb[:], encT_rhs.rearrange("b (k p) s -> p b k s", p=128)
            )
            enc_sb = wpool.tile([128, NB, SC, H], BF, tag="enc")
            nc.sync.dma_start(
                enc_sb[:], enc_lhsT.rearrange("b (k p) h -> p b k h", p=128)
            )
            maskT_sb = wpool.tile([128, NB, SC], F32, tag="maskT")
            nc.sync.dma_start(maskT_sb[:], maskT_in.rearrange("b p k -> p b k"))
            valid_sb = wpool.tile([128, NB, TG], F32, tag="valid")
            nc.sync.dma_start(valid_sb[:], valid_in.rearrange("b p k -> p b k"))

            # dec accumulator: [128, group, hk, t_in_group, batch]
            dec_all = wpool.tile([128, TG, HC, 128, NB], BF, tag="dec_all")
            # enc_proj^T per post batch: [128, b, hk, S]
            epT_sb = wpool.tile([128, NB, HC, S], BF, tag="epT")

            # ---------- prologue: ih half0 for iteration 0 (xh == 0) ----------
            emb_t = spool.tile([128, HC, COLS], BF, tag="emb_t")
            nc.sync.dma_start(emb_t[:], emb_stream[0].rearrange("(k p) c -> p k c", p=128))
            ih_cur = spool.tile([128, MC, COLS], BF, tag="ih")

            def ih0_mchunk(m, half, emb_tile, ih_tile):
                c0, c1 = half * HCOL, (half + 1) * HCOL

                def run():
                    ps = pih.tile([128, HCOL], F32, tag="ihps")
                    for kk in range(HC):
                        nc.tensor.matmul(
                            ps[:], wemb_sb[:, kk, m * 128:(m + 1) * 128],
                            emb_tile[:, kk, c0:c1], start=(kk == 0), stop=(kk == HC - 1),
                        )
                    nc.scalar.add(ih_tile[:, m, c0:c1], ps[:], biasp_sb[:, m:m + 1])
                return run

            for m in range(MC):
                ih0_mchunk(m, 0, emb_t, ih_cur)()

            def epT_thunk(b, hk):
                def run():
                    pp = ppost.tile([128, S], F32, tag="pp")
                    for e in range(HC):
                        nc.tensor.matmul(
                            pp[:], wattT_sb[:, e, hk * 128:(hk + 1) * 128],
                            encT_sb[:, b, e, :], start=(e == 0), stop=(e == HC - 1),
                        )
                    nc.scalar.add(epT_sb[:, b, hk, :], pp[:], batt_sb[:, hk:hk + 1])
                return run

            # =====================================================
            # helper closures
            # =====================================================
            def make_ih_half(k_iter, half, emb_tile, ih_tile, ag_src):
                """Thunks for ih of iteration k_iter, half columns; xh from ag_src."""
                thunks = []
                xh_box = {}

                def load_xh():
                    xh = spool.tile([128, 2 * HC, HCOL], BF, tag=f"xh{half}")
                    if ag_src is None:
                        nc.vector.memset(xh[:], 0.0)
                    else:
                        nc.sync.dma_start(
                            xh[:], ag_src[0:2].rearrange("s (k p) c -> p (s k) c", p=128)
                        )
                    xh_box["t"] = xh

                thunks.append(load_xh)
                c0, c1 = half * HCOL, (half + 1) * HCOL

                def mchunk(m):
                    def run():
                        xh = xh_box["t"]
                        ps = pih.tile([128, HCOL], F32, tag="ihps")
                        for kk in range(HC):
                            nc.tensor.matmul(
                                ps[:], wemb_sb[:, kk, m * 128:(m + 1) * 128],
                                emb_tile[:, kk, c0:c1], start=(kk == 0), stop=False,
                            )
                        for kk in range(2 * HC):
                            nc.tensor.matmul(
                                ps[:], win_sb[:, kk, m * 128:(m + 1) * 128],
                                xh[:, kk, :], start=False, stop=(kk == 2 * HC - 1),
                            )
                        nc.scalar.add(ih_tile[:, m, c0:c1], ps[:], biasp_sb[:, m:m + 1])
                    return run

                for m in range(MC):
                    thunks.append(mchunk(m))
                return thunks

            def make_dec_half(chunk, half, accum_src):
                """Thunks: ReduceScatter the masked L1 h half-chunk so each core
                receives dec = h1f + h1b for ITS post batches. Cores with
                gamma=0 contribute zeros; cores 2,3 (L1f/L1b) contribute their
                h sliced per receiver batch; add happens in the collective."""
                g, j = chunk // 4, chunk % 4
                pos = j * 32 + half * 16
                c0 = (1 + half * 16) * B
                box = {}

                def send():
                    # b-major masked copy so per-receiver slices are contiguous
                    hm = spool.tile([128, HC, B, 16], BF, tag="hm", bufs=1)
                    nc.gpsimd.tensor_scalar_mul(
                        hm[:],
                        accum_src[:, :, c0:c0 + HCOL].rearrange(
                            "p k (s b) -> p k b s", b=B
                        ),
                        gamma_sb[:, 0:1],
                    )
                    rs_in = dpool.tile([GS, H, NB * 16], BF, tag="rs_in")
                    for j2 in range(GS):
                        nc.sync.dma_start(
                            rs_in[j2].rearrange("(k p) (b s) -> p k b s", p=128, b=NB),
                            hm[:, :, NB * j2:NB * (j2 + 1), :],
                        )
                    rs_out = dpool.tile([H, NB * 16], BF, tag="rs_out")
                    nc.gpsimd.collective_compute(
                        "ReduceScatter", mybir.AluOpType.add, replica_groups=groups,
                        ins=[rs_in[:].opt()], outs=[rs_out[:].opt()],
                    )
                    box["t"] = rs_out

                def recv():
                    stg = spool.tile([128, HC, NB, 16], BF, tag="decstg", bufs=2)
                    nc.sync.dma_start(
                        stg[:], box["t"].rearrange("(k p) (b s) -> p k b s", p=128, b=NB)
                    )
                    nc.gpsimd.tensor_copy(
                        dec_all[:, g, :, pos:pos + 16, :],
                        stg.rearrange("p k b s -> p k s b"),
                    )

                return [send, recv]

            def make_post(g):
                """Thunks for post phase of t-group g (128 timesteps), NB batches."""
                thunks = []
                for b in range(NB):
                    state = {}

                    def score_sk(b, sk, state=state):
                        def run():
                            sp = ppost.tile([128, 128], F32, tag="pp")
                            for hk in range(HC):
                                nc.tensor.matmul(
                                    sp[:], epT_sb[:, b, hk, sk * 128:(sk + 1) * 128],
                                    dec_all[:, g, hk, :, b],
                                    start=(hk == 0), stop=(hk == HC - 1),
                                )
                            if sk == 0:
                                expT = spool.tile([128, SC, 128], BF, tag="expT", bufs=2)
                                state["expT"] = expT
                            nc.scalar.activation(
                                state["expT"][:, sk, :], sp[:],
                                mybir.ActivationFunctionType.Exp,
                                bias=maskT_sb[:, b, sk:sk + 1],
                            )
                        return run

                    def norm(b, state=state):
                        def run():
                            expT = state["expT"]
                            se = ppost.tile([1, 128], F32, tag="pp")
                            for sk in range(SC):
                                nc.tensor.matmul(
                                    se[:], ones_p[:, 0:1], expT[:, sk, :],
                                    start=(sk == 0), stop=(sk == SC - 1),
                                )
                            recip = steppool.tile([1, 128], BF, tag="recip")
                            with nc.allow_low_precision(reason="bf16 attention normalizer"):
                                nc.vector.reciprocal(recip[:], se[:])
                            bc = ppost.tile([128, 128], F32, tag="pp")
                            nc.tensor.matmul(
                                bc[:], ones_r[:, :], recip[:], start=True, stop=True,
                            )
                            att = spool.tile([128, SC, 128], BF, tag="att", bufs=2)
                            for sk in range(SC):
                                nc.vector.tensor_mul(att[:, sk, :], expT[:, sk, :], bc[:])
                            state["att"] = att
                        return run

                    def ctx_hk(b, hk, state=state):
                        def run():
                            att = state["att"]
                            cp = ppost.tile([128, 128], F32, tag="pp")
                            for sk in range(SC):
                                nc.tensor.matmul(
                                    cp[:], enc_sb[:, b, sk, hk * 128:(hk + 1) * 128],
                                    att[:, sk, :], start=(sk == 0), stop=(sk == SC - 1),
                                )
                            if hk == 0:
                                ctxT = spool.tile([128, HC, 128], BF, tag="ctxT", bufs=2)
                                state["ctxT"] = ctxT
                            nc.vector.tensor_copy(state["ctxT"][:, hk, :], cp[:])
                        return run

                    def fc_half(b, nh, state=state):
                        def run():
                            ctxT = state["ctxT"]
                            fp = ppost.tile([128, NH], F32, tag="pp")
                            for kk in range(2 * HC):
                                lhs = (dec_all[:, g, kk, :, b] if kk < HC
                                       else ctxT[:, kk - HC, :])
                                nc.tensor.matmul(
                                    fp[:], lhs, wfc_sb[:, kk, nh * NH:(nh + 1) * NH],
                                    start=(kk == 0), stop=False,
                                )
                            nc.tensor.matmul(
                                fp[:], ones_r[:, :], bfc_sb[:, nh * NH:(nh + 1) * NH],
                                start=False, stop=True,
                            )
                            osb = steppool.tile([128, NH], F32, tag="osb", bufs=2)
                            nc.vector.tensor_scalar_mul(osb[:], fp[:], valid_sb[:, b, g:g + 1])
                            nc.sync.dma_start(
                                out_d[b, g * 128:(g + 1) * 128, nh * NH:(nh + 1) * NH],
                                osb[:],
                            )
                        return run

                    for sk in range(SC):
                        thunks.append(score_sk(b, sk))
                    thunks.append(norm(b))
                    for hk in range(HC):
                        thunks.append(ctx_hk(b, hk))
                    for nh in range(2):
                        thunks.append(fc_half(b, nh))
                return thunks

            # =====================================================
            # main scan
            # =====================================================
            accum_prev = None
            c_cur = None
            emb_next = None
            ih_next = None
            # iter-0 gap work: ih0 half1 (emb-only) + enc_proj^T build
            pend_half1 = [ih0_mchunk(m, 1, emb_t, ih_cur) for m in range(MC)]
            pend_post = [epT_thunk(b, hk) for b in range(NB) for hk in range(HC)]

            for k in range(iters):
                # fresh per-iteration stream tiles
                if k > 0:
                    emb_t = emb_next
                    ih_cur = ih_next
                if k + 1 < iters:
                    emb_next = spool.tile([128, HC, COLS], BF, tag="emb_t")
                    nc.sync.dma_start(
                        emb_next[:], emb_stream[k + 1].rearrange("(k p) c -> p k c", p=128)
                    )
                    ih_next = spool.tile([128, MC, COLS], BF, tag="ih")

                # ---------- state carry / blend ----------
                accum = spool.tile([128, HC, (CH + 1) * B], BF, tag="accum")
                if k == 0:
                    nc.vector.tensor_copy(accum[:, :, 0:B], hinit_sb[:])
                    c_new0 = steppool.tile([128, HC, B], F32, tag="c")
                    nc.vector.tensor_copy(c_new0[:], cinit_sb[:])
                    c_cur = c_new0
                elif k == 1:
                    t1 = steppool.tile([128, HC, B], F32, tag="blend")
                    nc.vector.tensor_scalar_mul(t1[:], accum_prev[:, :, CH * B:], alpha_sb[:, 0:1])
                    t2 = steppool.tile([128, HC, B], F32, tag="blend")
                    nc.vector.tensor_scalar_mul(t2[:], hinit_sb[:], beta_sb[:, 0:1])
                    nc.vector.tensor_add(accum[:, :, 0:B], t1[:], t2[:])
                    t3 = steppool.tile([128, HC, B], F32, tag="blend")
                    nc.vector.tensor_scalar_mul(t3[:], c_cur[:], alpha_sb[:, 0:1])
                    t4 = steppool.tile([128, HC, B], F32, tag="blend")
                    nc.vector.tensor_scalar_mul(t4[:], cinit_sb[:], beta_sb[:, 0:1])
                    c_new1 = steppool.tile([128, HC, B], F32, tag="c")
                    nc.vector.tensor_add(c_new1[:], t3[:], t4[:])
                    c_cur = c_new1
                else:
                    nc.vector.tensor_copy(accum[:, :, 0:B], accum_prev[:, :, CH * B:])

                # ---------- filler schedule for this iteration ----------
                # fillers[s] = list of thunks issued right after step s's matmuls
                fillers = [[] for _ in range(CH)]
                for i, t in enumerate(pend_half1):
                    fillers[min(4 + i, 15)].append(t)
                for i, t in enumerate(pend_post):
                    fillers[min(8 + i, CH - 2)].append(t)
                pend_half0 = []
                if k + 1 < iters:
                    pend_half0 = make_ih_half(k + 1, 0, emb_next, ih_next, ag_out1)
                if k >= 1:
                    pend_half0 = make_dec_half(k - 1, 0, accum) + pend_half0
                for i, t in enumerate(pend_half0):
                    fillers[min(19 + i, CH - 1)].append(t)

                # ---------- CH recurrence steps ----------
                for s in range(CH):
                    if s == 16:
                        ag_in1 = dpool.tile([H, HCOL], BF, tag="ag_in1")
                        nc.sync.dma_start(
                            ag_in1.rearrange("(k p) c -> p k c", p=128),
                            accum[:, :, B:(1 + 16) * B],
                        )
                        nc.gpsimd.collective_compute(
                            "AllGather", mybir.AluOpType.bypass, replica_groups=groups,
                            ins=[ag_in1.opt()], outs=[ag_out1[:].opt()],
                        )

                    g_ps = pg.tile([128, MC, B], F32, tag="g")
                    # inject precomputed ih for this step into PSUM in one
                    # identity matmul (start=True clears), then accumulate the
                    # Whh contraction on top. Group shapes differ -> skip check.
                    nc.tensor.matmul(
                        g_ps[:], identity[:],
                        ih_cur[:, :, s * B:(s + 1) * B],
                        start=True, stop=False, skip_group_check=True,
                    )
                    # i,f (m 0..7) and g (m 12..15) first: the c-path chain can
                    # start while the o-gate matmuls (m 8..11) still stream.
                    for m in list(range(0, 8)) + list(range(12, 16)):
                        for kk in range(HC):
                            nc.tensor.matmul(
                                g_ps[:, m, :],
                                whh_sb[:, kk, m * 128:(m + 1) * 128],
                                accum[:, kk, s * B:(s + 1) * B],
                                start=False, stop=(kk == HC - 1),
                                skip_group_check=True,
                            )
                    sig = steppool.tile([128, 8, B], F32, tag="sig")
                    nc.scalar.activation(sig[:], g_ps[:, 0:8, :], mybir.ActivationFunctionType.Sigmoid)
                    tg = steppool.tile([128, HC, B], F32, tag="tg")
                    nc.scalar.activation(tg[:], g_ps[:, 12:16, :], mybir.ActivationFunctionType.Tanh)
                    for m in range(8, 12):
                        for kk in range(HC):
                            nc.tensor.matmul(
                                g_ps[:, m, :],
                                whh_sb[:, kk, m * 128:(m + 1) * 128],
                                accum[:, kk, s * B:(s + 1) * B],
                                start=False, stop=(kk == HC - 1),
                                skip_group_check=True,
                            )
                    sig_o = steppool.tile([128, HC, B], F32, tag="sig_o")
                    nc.scalar.activation(sig_o[:], g_ps[:, 8:12, :], mybir.ActivationFunctionType.Sigmoid)

                    m1 = steppool.tile([128, HC, B], F32, tag="m1")
                    nc.vector.tensor_mul(m1[:], sig[:, 4:8, :], c_cur[:])
                    m2 = steppool.tile([128, HC, B], F32, tag="m2")
                    nc.vector.tensor_mul(m2[:], sig[:, 0:4, :], tg[:])
                    c_new = steppool.tile([128, HC, B], F32, tag="c")
                    nc.vector.tensor_add(c_new[:], m1[:], m2[:])
                    tc_t = steppool.tile([128, HC, B], F32, tag="tc")
                    nc.scalar.activation(tc_t[:], c_new[:], mybir.ActivationFunctionType.Tanh)
                    nc.vector.tensor_mul(accum[:, :, (s + 1) * B:(s + 2) * B], sig_o[:], tc_t[:])
                    c_cur = c_new

                    for t in fillers[s]:
                        t()

                # ---------- second-half exchange ----------
                ag_in2 = dpool.tile([H, HCOL], BF, tag="ag_in2")
                nc.sync.dma_start(
                    ag_in2.rearrange("(k p) c -> p k c", p=128),
                    accum[:, :, (1 + 16) * B:(1 + 32) * B],
                )
                nc.gpsimd.collective_compute(
                    "AllGather", mybir.AluOpType.bypass, replica_groups=groups,
                    ins=[ag_in2.opt()], outs=[ag_out2[:].opt()],
                )

                # ---------- queue work that depends on AG2_k ----------
                pend_half1 = []
                if k + 1 < iters:
                    pend_half1 = make_ih_half(k + 1, 1, emb_next, ih_next, ag_out2)
                if k >= 1:
                    pend_half1 = make_dec_half(k - 1, 1, accum) + pend_half1
                pend_post = []
                if k + 1 >= 6 and (k + 1 - 6) % 4 == 0 and (k + 1 - 6) // 4 < 3:
                    pend_post = make_post((k + 1 - 6) // 4)

                accum_prev = accum

            # ---------- tail: dec half1 of chunk 15 + post group 3 ----------
            for t in pend_half1:
                t()
            for t in make_post(3):
                t()

    nc.compile()
    return nc


# ---------------- host-side preparation ----------------

def _prep_inputs(inputs, nch=NCH):
    assert nch == NCH
    iters = ITERS
    perm = _gate_perm()

    trg = np.asarray(inputs["trg_inputs"]).astype(np.int64)
    trg_len = np.asarray(inputs["trg_len"]).astype(np.int64)
    enc = _f32(inputs["encoder_outputs"])
    h0 = _f32(inputs["h0"]).reshape(L, 2, B, H)
    c0 = _f32(inputs["c0"]).reshape(L, 2, B, H)
    embed = _f32(inputs["embed"])
    W_ih0 = _f32(inputs["W_ih0"])          # [2, 4H, E]
    W_ih1 = _f32(inputs["W_ih1"])[0]       # [2, 4H, 2H]
    W_hh = _f32(inputs["W_hh"])            # [L, 2, 4H, H]
    b_ih = _f32(inputs["b_ih"])            # [L, 2, 4H]
    b_hh = _f32(inputs["b_hh"])
    W_att = _f32(inputs["W_att"])          # [H, H]
    b_att = _f32(inputs["b_att"])          # [H]
    W_fc = _f32(inputs["W_fc"])            # [OUT, 2H]
    b_fc = _f32(inputs["b_fc"])            # [OUT]

    # embedding stream  [iters, E, COLS]; emb_stream[k,e,s*B+b] = X[b,32k+s,e]
    X = embed[trg[:, :T]]                             # [B, T, E]
    es = np.zeros((iters, E, COLS), np.float32)
    xt = X.transpose(2, 1, 0)                         # [E, T, B]
    es[:NCH] = (
        xt.reshape(E, NCH, CH, B).transpose(1, 0, 2, 3).reshape(NCH, E, COLS)
    )
    es = _bf(es)

    cells = [(0, 0), (0, 1), (1, 0), (1, 1)]          # (layer, dir)
    zeros_emb = _bf(np.zeros((E, 4 * H)))
    zeros_in = _bf(np.zeros((2 * H, 4 * H)))

    # masks in transposed layouts
    # maskT [B, 128, SC]: 0 where s < len else -1e30  (s = sk*128 + p)
    # -30 shift guards exp() against fp32 overflow (scores can reach ~90);
    # the shift cancels in the softmax normalization.
    s_idx = np.arange(S).reshape(SC, 128).T           # [128, SC]
    maskT = np.where(s_idx[None, :, :] < trg_len[:, None, None], -30.0, -1e30).astype(np.float32)
    # validT [B, 128, TG]: 1 where t < len else 0  (t = g*128 + p)
    t_idx = np.arange(T).reshape(TG, 128).T
    validT = (t_idx[None, :, :] < trg_len[:, None, None]).astype(np.float32)

    encT = enc.transpose(0, 2, 1)                     # [B, H, S]

    in_maps = []
    for c in range(N_CORES):
        cell = c % 4
        layer, d = cells[cell]
        if layer == 0:
            wemb = _bf(W_ih0[d][perm].T)              # [E, 4H]
            win = zeros_in
        else:
            wemb = zeros_emb
            win = _bf(W_ih1[d][perm].T)               # [2H, 4H]
        whh = _bf(W_hh[layer, d][perm].T)             # [H, 4H]
        bp = (b_ih[layer, d] + b_hh[layer, d])[perm]  # [4H]
        biasp = _f32(bp.reshape(MC, 128).T)           # [128, MC]
        hin = h0[layer, d]                            # [B, H]
        cin = c0[layer, d]
        h_init = _bf(hin.T.reshape(HC, 128, B).transpose(1, 0, 2))   # [128,HC,B]
        c_init = _f32(cin.T.reshape(HC, 128, B).transpose(1, 0, 2))
        a = 1.0 if layer == 0 else 0.0
        alpha = _f32(np.full((128, 1), a))
        beta = _f32(np.full((128, 1), 1.0 - a))
        gam = _f32(np.full((128, 1), 1.0 if c in (2, 3) else 0.0))

        bsl = [2 * c, 2 * c + 1]                      # post batches of this core

        m = dict(
            w_emb=wemb, w_in=win, w_hh=whh, biasp=biasp,
            h_init=h_init, c_init=c_init, alpha=alpha, beta=beta, gamma=gam,
            emb_stream=es,
            enc_lhsT=_bf(enc[bsl]),                   # [2, S, H]
            encT_rhs=_bf(encT[bsl]),                  # [2, H, S]
            w_attT=_bf(W_att.T),
            b_att_in=_f32(b_att.reshape(HC, 128).T),
            maskT_in=_f32(maskT[bsl]),                # [2,128,SC]
            valid_in=_f32(validT[bsl]),               # [2,128,TG]
            w_fcT=_bf(W_fc.T),                        # [2H, OUT]
            b_fc_row=_bf(b_fc[None, :]),
        )
        in_maps.append(m)
    return in_maps


_NC_CACHE = {}


def kernel(**inputs) -> np.ndarray:
    nch = int(os.environ.get("KERNEL_NCH", NCH))
    if nch not in _NC_CACHE:
        _NC_CACHE[nch] = build_nc(nch)
    nc = _NC_CACHE[nch]
    in_maps = _prep_inputs(inputs, nch)
    r = run_bass_kernel_spmd(nc, in_maps, list(range(N_CORES)))
    outs = [np.asarray(r.results[c]["out"], np.float32) for c in range(N_CORES)]
    return np.concatenate(outs, axis=0)


# revision 50
# speedup vs baseline: 1.2695x; 1.1506x over previous
"""Bass/Trainium2 kernel for nn_Decoder (2-layer bidir-style LSTM decoder
with general attention + fc), distributed over 8 NeuronCores.

v2 architecture (SPMD, one uniform program; per-core behavior differs only
in input DATA):
  - 4 LSTM cells (L0f, L0b, L1f, L1b) -> cores 0..3; cores 4..7 mirror the
    cells but own DIFFERENT post-phase batches. Scan is chunked, CH=32
    steps; L1 runs one chunk behind L0.
  - AllGather is split into two half-chunk collectives per iteration
    (issued after step 15 / step 31) over 4-rank groups, so transfer time
    hides under the recurrence.
  - ih precompute (W_emb @ emb + W_in @ xh + b) runs as two half-chunk
    passes interleaved into the per-step nonlinearity gaps of the
    surrounding iterations (keeps PE warm, off critical path).
  - Per step: gates = Whh @ h (4 k-chunks) + identity-matmul injection of
    the precomputed ih column (start=False accumulate), so the sigmoid
    reads PSUM directly and the old DVE add + sync gap disappears.
  - Post phase (attention + fc) is batch-split: core c handles batches
    2c, 2c+1 only, in transposed-score form:
      scoreT[s,t] = epT-tiles @ decT, exp(score + maskT) with the ragged
      mask folded into the ACT bias (no max subtraction; fp32 range is
      sufficient), sumexp via ones-matmul partition reduction, attention
      normalized through an outer-product broadcast matmul, ctxT and fc
      as dense matmuls with bias injected by rank-1 matmuls.
    Post work for chunk-group g is interleaved into iteration 4g+6's step
    gaps (g=3 is a short tail). Host stitches the 8 per-core outputs.

Numerics: bf16 weights/activations, fp32 PSUM + fp32 cell state c.
"""

import os
import sys

sys.path.insert(0, "/opt/trn_rl_repo")

import numpy as np
import ml_dtypes

import concourse.bass as bass
import concourse.mybir as mybir
import concourse.tile as tile
from concourse import bacc
from concourse.bass_utils import run_bass_kernel_spmd
from concourse.masks import make_identity

# ---- problem constants (hardcoded per contract) ----
L = 2
H = 512
E = 512
B = 16
T = 512
S = 512
VOCAB = 1001
OUT = 1000

N_CORES = 8
GS = 8                        # AllGather replica-group size (8-rank world group)
CH = 32                       # timesteps per chunk
NCH = T // CH                 # 16 chunks
ITERS = NCH + 1               # L1 lags one chunk
COLS = CH * B                 # 512 columns per chunk (s-major, b-minor)
HCOL = COLS // 2              # 256 columns per half chunk
HC = H // 128                 # 4 H-chunks
MC = (4 * H) // 128           # 16 gate M-chunks
SC = S // 128                 # 4 S-chunks
TG = 4                        # t-groups for post (128 steps each)
NB = 2                        # batches per core in post phase
NH = OUT // 2                 # fc output half width (psum bank limit)
BF = mybir.dt.bfloat16
F32 = mybir.dt.float32


# gate order: keep torch's i,f,g,o — the c-path gates (i,f,g) are then the
# contiguous first 12 m-chunks and o is the last 4, so each of the two gate
# PSUM tiles is initialized by exactly one full-coverage start=True inject.
def _gate_perm():
    return np.arange(4 * H)


def _bf(x):
    return np.ascontiguousarray(np.asarray(x, dtype=np.float32)).astype(ml_dtypes.bfloat16)


def _f32(x):
    return np.ascontiguousarray(np.asarray(x, dtype=np.float32))


def build_nc(nch=NCH):
    assert nch == NCH, "v2 kernel supports full problem size only"
    iters = ITERS
    nc = bacc.Bacc("TRN2", target_bir_lowering=False, debug=False, num_devices=N_CORES)

    # ---- DRAM inputs ----
    w_emb = nc.dram_tensor("w_emb", [E, 4 * H], BF, kind="ExternalInput")
    w_in = nc.dram_tensor("w_in", [2 * H, 4 * H], BF, kind="ExternalInput")
    w_hh = nc.dram_tensor("w_hh", [H, 4 * H], BF, kind="ExternalInput")
    biasp = nc.dram_tensor("biasp", [128, MC], F32, kind="ExternalInput")
    h_init = nc.dram_tensor("h_init", [128, HC, B], BF, kind="ExternalInput")
    c_init = nc.dram_tensor("c_init", [128, HC, B], F32, kind="ExternalInput")
    alpha = nc.dram_tensor("alpha", [128, 1], F32, kind="ExternalInput")
    beta = nc.dram_tensor("beta", [128, 1], F32, kind="ExternalInput")
    emb_stream = nc.dram_tensor("emb_stream", [iters, E, COLS], BF, kind="ExternalInput")
    enc_lhsT = nc.dram_tensor("enc_lhsT", [NB, S, H], BF, kind="ExternalInput")
    encT_rhs = nc.dram_tensor("encT_rhs", [NB, H, S], BF, kind="ExternalInput")
    w_attT = nc.dram_tensor("w_attT", [E, H], BF, kind="ExternalInput")
    b_att_in = nc.dram_tensor("b_att_in", [128, HC], F32, kind="ExternalInput")
    maskT_in = nc.dram_tensor("maskT_in", [NB, 128, SC], F32, kind="ExternalInput")
    valid_in = nc.dram_tensor("valid_in", [NB, 128, TG], F32, kind="ExternalInput")
    w_fcT = nc.dram_tensor("w_fcT", [2 * H, OUT], BF, kind="ExternalInput")
    b_fc_row = nc.dram_tensor("b_fc_row", [1, OUT], BF, kind="ExternalInput")
    out_d = nc.dram_tensor("out", [NB, T, OUT], F32, kind="ExternalOutput")

    # ---- DRAM internals (collective bounce buffers) ----
    ag_out1 = nc.dram_tensor("ag_out1", [GS, H, HCOL], BF, addr_space="Shared")
    ag_out2 = nc.dram_tensor("ag_out2", [GS, H, HCOL], BF, addr_space="Shared")
    gamma = nc.dram_tensor("gamma", [128, 1], F32, kind="ExternalInput")

    if GS == 4:
        groups = [[0, 1, 2, 3], [4, 5, 6, 7]]
    else:
        groups = [list(range(N_CORES))]

    with tile.TileContext(nc) as tc:
        with (
            tc.tile_pool(name="wpool", bufs=1) as wpool,
            tc.tile_pool(name="spool", bufs=2) as spool,
            tc.tile_pool(name="steppool", bufs=3) as steppool,
            tc.tile_pool(name="pg", bufs=2, space="PSUM") as pg,
            tc.tile_pool(name="pih", bufs=2, space="PSUM") as pih,
            tc.tile_pool(name="ppost", bufs=2, space="PSUM") as ppost,
            tc.tile_pool(name="dpool", bufs=2, space="DRAM") as dpool,
        ):
            # ---- persistent SBUF ----
            wemb_sb = wpool.tile([128, HC, 4 * H], BF, tag="wemb")
            nc.sync.dma_start(wemb_sb[:], w_emb.rearrange("(k p) m -> p k m", p=128))
            win_sb = wpool.tile([128, 2 * HC, 4 * H], BF, tag="win")
            nc.sync.dma_start(win_sb[:], w_in.rearrange("(k p) m -> p k m", p=128))
            whh_sb = wpool.tile([128, HC, 4 * H], BF, tag="whh")
            nc.sync.dma_start(whh_sb[:], w_hh.rearrange("(k p) m -> p k m", p=128))
            biasp_sb = wpool.tile([128, MC], F32, tag="biasp")
            nc.sync.dma_start(biasp_sb[:], biasp[:])
            hinit_sb = wpool.tile([128, HC, B], BF, tag="hinit")
            nc.sync.dma_start(hinit_sb[:], h_init[:])
            cinit_sb = wpool.tile([128, HC, B], F32, tag="cinit")
            nc.sync.dma_start(cinit_sb[:], c_init[:])
            alpha_sb = wpool.tile([128, 1], F32, tag="alpha")
            nc.sync.dma_start(alpha_sb[:], alpha[:])
            beta_sb = wpool.tile([128, 1], F32, tag="beta")
            nc.sync.dma_start(beta_sb[:], beta[:])
            gamma_sb = wpool.tile([128, 1], F32, tag="gamma")
            nc.sync.dma_start(gamma_sb[:], gamma[:])

            identity = wpool.tile([128, 128], BF, tag="ident")
            make_identity(nc, identity[:])
            ones_r = wpool.tile([1, 128], BF, tag="ones_r")
            nc.vector.memset(ones_r[:], 1.0)
            ones_p = wpool.tile([128, 1], BF, tag="ones_p")
            nc.vector.memset(ones_p[:], 1.0)

            wattT_sb = wpool.tile([128, HC, H], BF, tag="wattT")
            nc.sync.dma_start(wattT_sb[:], w_attT.rearrange("(k p) m -> p k m", p=128))
            batt_sb = wpool.tile([128, HC], F32, tag="batt")
            nc.sync.dma_start(batt_sb[:], b_att_in[:])
            wfc_sb = wpool.tile([128, 2 * HC, OUT], BF, tag="wfc")
            nc.sync.dma_start(wfc_sb[:], w_fcT.rearrange("(k p) m -> p k m", p=128))
            bfc_sb = wpool.tile([1, OUT], BF, tag="bfc")
            nc.sync.dma_start(bfc_sb[:], b_fc_row[:])
            encT_sb = wpool.tile([128, NB, HC, S], BF, tag="encT")
            nc.sync.dma_start(
                encT_sb[:], encT_rhs.rearrange("b (k p) s -> p b k s", p=128)
            )
            enc_sb = wpool.tile([128, NB, SC, H], BF, tag="enc")
            nc.sync.dma_start(
                enc_sb[:], enc_lhsT.rearrange("b (k p) h -> p b k h", p=128)
            )
            maskT_sb = wpool.tile([128, NB, SC], F32, tag="maskT")
            nc.sync.dma_start(maskT_sb[:], maskT_in.rearrange("b p k -> p b k"))
            valid_sb = wpool.tile([128, NB, TG], F32, tag="valid")
            nc.sync.dma_start(valid_sb[:], valid_in.rearrange("b p k -> p b k"))

            # dec accumulator: [128, group, hk, t_in_group, batch]
            dec_all = wpool.tile([128, TG, HC, 128, NB], BF, tag="dec_all")
            # enc_proj^T per post batch: [128, b, hk, S]
            epT_sb = wpool.tile([128, NB, HC, S], BF, tag="epT")

            # ---------- prologue: ih half0 for iteration 0 (xh == 0) ----------
            emb_t = spool.tile([128, HC, COLS], BF, tag="emb_t")
            nc.sync.dma_start(emb_t[:], emb_stream[0].rearrange("(k p) c -> p k c", p=128))
            ih_cur = spool.tile([128, MC, COLS], BF, tag="ih")

            def ih0_mchunk(m, half, emb_tile, ih_tile):
                c0, c1 = half * HCOL, (half + 1) * HCOL

                def run():
                    ps = pih.tile([128, HCOL], F32, tag="ihps")
                    for kk in range(HC):
                        nc.tensor.matmul(
                            ps[:], wemb_sb[:, kk, m * 128:(m + 1) * 128],
                            emb_tile[:, kk, c0:c1], start=(kk == 0), stop=(kk == HC - 1),
                        )
                    nc.scalar.add(ih_tile[:, m, c0:c1], ps[:], biasp_sb[:, m:m + 1])
                return run

            for m in range(MC):
                ih0_mchunk(m, 0, emb_t, ih_cur)()

            def epT_thunk(b, hk):
                def run():
                    pp = ppost.tile([128, S], F32, tag="pp")
                    for e in range(HC):
                        nc.tensor.matmul(
                            pp[:], wattT_sb[:, e, hk * 128:(hk + 1) * 128],
                            encT_sb[:, b, e, :], start=(e == 0), stop=(e == HC - 1),
                        )
                    nc.scalar.add(epT_sb[:, b, hk, :], pp[:], batt_sb[:, hk:hk + 1])
                return run

            # =====================================================
            # helper closures
            # =====================================================
            def make_ih_half(k_iter, half, emb_tile, ih_tile, ag_src):
                """Thunks for ih of iteration k_iter, half columns; xh from ag_src."""
                thunks = []
                xh_box = {}

                def load_xh():
                    xh = spool.tile([128, 2 * HC, HCOL], BF, tag=f"xh{half}")
                    if ag_src is None:
                        nc.vector.memset(xh[:], 0.0)
                    else:
                        nc.sync.dma_start(
                            xh[:], ag_src[0:2].rearrange("s (k p) c -> p (s k) c", p=128)
                        )
                    xh_box["t"] = xh

                thunks.append(load_xh)
                c0, c1 = half * HCOL, (half + 1) * HCOL

                def mchunk(m):
                    def run():
                        xh = xh_box["t"]
                        ps = pih.tile([128, HCOL], F32, tag="ihps")
                        for kk in range(HC):
                            nc.tensor.matmul(
                                ps[:], wemb_sb[:, kk, m * 128:(m + 1) * 128],
                                emb_tile[:, kk, c0:c1], start=(kk == 0), stop=False,
                            )
                        for kk in range(2 * HC):
                            nc.tensor.matmul(
                                ps[:], win_sb[:, kk, m * 128:(m + 1) * 128],
                                xh[:, kk, :], start=False, stop=(kk == 2 * HC - 1),
                            )
                        nc.scalar.add(ih_tile[:, m, c0:c1], ps[:], biasp_sb[:, m:m + 1])
                    return run

                for m in range(MC):
                    thunks.append(mchunk(m))
                return thunks

            def make_dec_half(chunk, half, accum_src):
                """Thunks: ReduceScatter the masked L1 h half-chunk so each core
                receives dec = h1f + h1b for ITS post batches. Cores with
                gamma=0 contribute zeros; cores 2,3 (L1f/L1b) contribute their
                h sliced per receiver batch; add happens in the collective."""
                g, j = chunk // 4, chunk % 4
                pos = j * 32 + half * 16
                c0 = (1 + half * 16) * B
                box = {}

                def send():
                    # b-major masked copy so per-receiver slices are contiguous
                    hm = spool.tile([128, HC, B, 16], BF, tag="hm", bufs=1)
                    nc.vector.tensor_scalar_mul(
                        hm[:],
                        accum_src[:, :, c0:c0 + HCOL].rearrange(
                            "p k (s b) -> p k b s", b=B
                        ),
                        gamma_sb[:, 0:1],
                    )
                    rs_in = dpool.tile([GS, H, NB * 16], BF, tag="rs_in")
                    for j2 in range(GS):
                        nc.sync.dma_start(
                            rs_in[j2].rearrange("(k p) (b s) -> p k b s", p=128, b=NB),
                            hm[:, :, NB * j2:NB * (j2 + 1), :],
                        )
                    rs_out = dpool.tile([H, NB * 16], BF, tag="rs_out")
                    nc.gpsimd.collective_compute(
                        "ReduceScatter", mybir.AluOpType.add, replica_groups=groups,
                        ins=[rs_in[:].opt()], outs=[rs_out[:].opt()],
                    )
                    box["t"] = rs_out

                def recv():
                    stg = spool.tile([128, HC, NB, 16], BF, tag="decstg", bufs=2)
                    nc.sync.dma_start(
                        stg[:], box["t"].rearrange("(k p) (b s) -> p k b s", p=128, b=NB)
                    )
                    nc.vector.tensor_copy(
                        dec_all[:, g, :, pos:pos + 16, :],
                        stg.rearrange("p k b s -> p k s b"),
                    )

                return [send, recv]

            def make_post(g):
                """Thunks for post phase of t-group g (128 timesteps), NB batches."""
                thunks = []
                for b in range(NB):
                    state = {}

                    def score_sk(b, sk, state=state):
                        def run():
                            sp = ppost.tile([128, 128], F32, tag="pp")
                            for hk in range(HC):
                                nc.tensor.matmul(
                                    sp[:], epT_sb[:, b, hk, sk * 128:(sk + 1) * 128],
                                    dec_all[:, g, hk, :, b],
                                    start=(hk == 0), stop=(hk == HC - 1),
                                )
                            if sk == 0:
                                expT = spool.tile([128, SC, 128], BF, tag="expT", bufs=2)
                                state["expT"] = expT
                            nc.scalar.activation(
                                state["expT"][:, sk, :], sp[:],
                                mybir.ActivationFunctionType.Exp,
                                bias=maskT_sb[:, b, sk:sk + 1],
                            )
                        return run

                    def norm(b, state=state):
                        def run():
                            expT = state["expT"]
                            se = ppost.tile([1, 128], F32, tag="pp")
                            for sk in range(SC):
                                nc.tensor.matmul(
                                    se[:], ones_p[:, 0:1], expT[:, sk, :],
                                    start=(sk == 0), stop=(sk == SC - 1),
                                )
                            recip = steppool.tile([1, 128], BF, tag="recip")
                            with nc.allow_low_precision(reason="bf16 attention normalizer"):
                                nc.vector.reciprocal(recip[:], se[:])
                            bc = ppost.tile([128, 128], F32, tag="pp")
                            nc.tensor.matmul(
                                bc[:], ones_r[:, :], recip[:], start=True, stop=True,
                            )
                            att = spool.tile([128, SC, 128], BF, tag="att", bufs=2)
                            for sk in range(SC):
                                nc.vector.tensor_mul(att[:, sk, :], expT[:, sk, :], bc[:])
                            state["att"] = att
                        return run

                    def ctx_hk(b, hk, state=state):
                        def run():
                            att = state["att"]
                            cp = ppost.tile([128, 128], F32, tag="pp")
                            for sk in range(SC):
                                nc.tensor.matmul(
                                    cp[:], enc_sb[:, b, sk, hk * 128:(hk + 1) * 128],
                                    att[:, sk, :], start=(sk == 0), stop=(sk == SC - 1),
                                )
                            if hk == 0:
                                ctxT = spool.tile([128, HC, 128], BF, tag="ctxT", bufs=2)
                                state["ctxT"] = ctxT
                            nc.vector.tensor_copy(state["ctxT"][:, hk, :], cp[:])
                        return run

                    def fc_half(b, nh, state=state):
                        def run():
                            ctxT = state["ctxT"]
                            fp = ppost.tile([128, NH], F32, tag="pp")
                            for kk in range(2 * HC):
                                lhs = (dec_all[:, g, kk, :, b] if kk < HC
                                       else ctxT[:, kk - HC, :])
                                nc.tensor.matmul(
                                    fp[:], lhs, wfc_sb[:, kk, nh * NH:(nh + 1) * NH],
                                    start=(kk == 0), stop=False,
                                )
                            nc.tensor.matmul(
                                fp[:], ones_r[:, :], bfc_sb[:, nh * NH:(nh + 1) * NH],
                                start=False, stop=True,
                            )
                            osb = steppool.tile([128, NH], F32, tag="osb", bufs=2)
                            nc.vector.tensor_scalar_mul(osb[:], fp[:], valid_sb[:, b, g:g + 1])
                            nc.sync.dma_start(
                                out_d[b, g * 128:(g + 1) * 128, nh * NH:(nh + 1) * NH],
                                osb[:],
                            )
                        return run

                    for sk in range(SC):
                        thunks.append(score_sk(b, sk))
                    thunks.append(norm(b))
                    for hk in range(HC):
                        thunks.append(ctx_hk(b, hk))
                    for nh in range(2):
                        thunks.append(fc_half(b, nh))
                return thunks

            # =====================================================
            # main scan
            # =====================================================
            accum_prev = None
            c_cur = None
            emb_next = None
            ih_next = None
            # iter-0 gap work: ih0 half1 (emb-only) + enc_proj^T build
            pend_half1 = [ih0_mchunk(m, 1, emb_t, ih_cur) for m in range(MC)]
            pend_post = [epT_thunk(b, hk) for b in range(NB) for hk in range(HC)]
            post_queue = []

            for k in range(iters):
                # fresh per-iteration stream tiles
                if k > 0:
                    emb_t = emb_next
                    ih_cur = ih_next
                if k + 1 < iters:
                    emb_next = spool.tile([128, HC, COLS], BF, tag="emb_t")
                    nc.sync.dma_start(
                        emb_next[:], emb_stream[k + 1].rearrange("(k p) c -> p k c", p=128)
                    )
                    ih_next = spool.tile([128, MC, COLS], BF, tag="ih")

                # ---------- state carry / blend ----------
                accum = spool.tile([128, HC, (CH + 1) * B], BF, tag="accum")
                if k == 0:
                    nc.vector.tensor_copy(accum[:, :, 0:B], hinit_sb[:])
                    c_new0 = steppool.tile([128, HC, B], F32, tag="c")
                    nc.vector.tensor_copy(c_new0[:], cinit_sb[:])
                    c_cur = c_new0
                elif k == 1:
                    t1 = steppool.tile([128, HC, B], F32, tag="blend")
                    nc.vector.tensor_scalar_mul(t1[:], accum_prev[:, :, CH * B:], alpha_sb[:, 0:1])
                    t2 = steppool.tile([128, HC, B], F32, tag="blend")
                    nc.vector.tensor_scalar_mul(t2[:], hinit_sb[:], beta_sb[:, 0:1])
                    nc.vector.tensor_add(accum[:, :, 0:B], t1[:], t2[:])
                    t3 = steppool.tile([128, HC, B], F32, tag="blend")
                    nc.vector.tensor_scalar_mul(t3[:], c_cur[:], alpha_sb[:, 0:1])
                    t4 = steppool.tile([128, HC, B], F32, tag="blend")
                    nc.vector.tensor_scalar_mul(t4[:], cinit_sb[:], beta_sb[:, 0:1])
                    c_new1 = steppool.tile([128, HC, B], F32, tag="c")
                    nc.vector.tensor_add(c_new1[:], t3[:], t4[:])
                    c_cur = c_new1
                else:
                    nc.vector.tensor_copy(accum[:, :, 0:B], accum_prev[:, :, CH * B:])

                # ---------- filler schedule for this iteration ----------
                # fillers[s] = list of thunks issued right after step s's matmuls
                fillers = [[] for _ in range(CH)]
                for i, t in enumerate(pend_half1):
                    fillers[min(5 + i, 15)].append(t)
                for i, t in enumerate(pend_post):
                    fillers[min(8 + i, CH - 2)].append(t)
                pend_half0 = []
                if k + 1 < iters:
                    pend_half0 = make_ih_half(k + 1, 0, emb_next, ih_next, ag_out1)
                if k >= 1:
                    pend_half0 = make_dec_half(k - 1, 0, accum) + pend_half0
                for i, t in enumerate(pend_half0):
                    fillers[min(19 + i, CH - 1)].append(t)

                # ---------- CH recurrence steps ----------
                for s in range(CH):
                    if s == 16:
                        ag_in1 = dpool.tile([H, HCOL], BF, tag="ag_in1")
                        nc.sync.dma_start(
                            ag_in1.rearrange("(k p) c -> p k c", p=128),
                            accum[:, :, B:(1 + 16) * B],
                        )
                        nc.gpsimd.collective_compute(
                            "AllGather", mybir.AluOpType.bypass, replica_groups=groups,
                            ins=[ag_in1.opt()], outs=[ag_out1[:].opt()],
                        )

                    # Separate PSUM tiles for the c-path gates (i,f,g) and the
                    # o gate so the o matmuls don't serialize behind the
                    # sigmoid/tanh reads of the c-path tile (tile-granular
                    # dependency tracking). ih injected per tile via identity
                    # matmuls; Whh accumulates on top (skip group checks).
                    g_ifg = pg.tile([128, 12, B], F32, tag="g")
                    g_o = pg.tile([128, HC, B], F32, tag="go")
                    nc.tensor.matmul(
                        g_ifg[:], identity[:],
                        ih_cur[:, 0:12, s * B:(s + 1) * B],
                        start=True, stop=False, skip_group_check=True,
                    )
                    # i,f,g (m 0..11) first: the c-path chain starts while the
                    # o-gate matmuls (m 12..15) still stream on PE.
                    for m in range(12):
                        for kk in range(HC):
                            nc.tensor.matmul(
                                g_ifg[:, m, :],
                                whh_sb[:, kk, m * 128:(m + 1) * 128],
                                accum[:, kk, s * B:(s + 1) * B],
                                start=False, stop=(kk == HC - 1),
                                skip_group_check=True,
                            )
                    sig = steppool.tile([128, 8, B], F32, tag="sig")
                    nc.scalar.activation(sig[:], g_ifg[:, 0:8, :], mybir.ActivationFunctionType.Sigmoid)
                    tg = steppool.tile([128, HC, B], F32, tag="tg")
                    nc.scalar.activation(tg[:], g_ifg[:, 8:12, :], mybir.ActivationFunctionType.Tanh)
                    nc.tensor.matmul(
                        g_o[:], identity[:],
                        ih_cur[:, 12:16, s * B:(s + 1) * B],
                        start=True, stop=False, skip_group_check=True,
                    )
                    for m in range(12, 16):
                        for kk in range(HC):
                            nc.tensor.matmul(
                                g_o[:, m - 12, :],
                                whh_sb[:, kk, m * 128:(m + 1) * 128],
                                accum[:, kk, s * B:(s + 1) * B],
                                start=False, stop=(kk == HC - 1),
                                skip_group_check=True,
                            )
                    sig_o = steppool.tile([128, HC, B], F32, tag="sig_o")
                    nc.scalar.activation(sig_o[:], g_o[:], mybir.ActivationFunctionType.Sigmoid)

                    m1 = steppool.tile([128, HC, B], F32, tag="m1")
                    nc.vector.tensor_mul(m1[:], sig[:, 4:8, :], c_cur[:])
                    m2 = steppool.tile([128, HC, B], F32, tag="m2")
                    nc.vector.tensor_mul(m2[:], sig[:, 0:4, :], tg[:])
                    c_new = steppool.tile([128, HC, B], F32, tag="c")
                    nc.vector.tensor_add(c_new[:], m1[:], m2[:])
                    tc_t = steppool.tile([128, HC, B], F32, tag="tc")
                    nc.scalar.activation(tc_t[:], c_new[:], mybir.ActivationFunctionType.Tanh)
                    nc.vector.tensor_mul(accum[:, :, (s + 1) * B:(s + 2) * B], sig_o[:], tc_t[:])
                    c_cur = c_new

                    for t in fillers[s]:
                        t()

                # ---------- second-half exchange ----------
                ag_in2 = dpool.tile([H, HCOL], BF, tag="ag_in2")
                nc.sync.dma_start(
                    ag_in2.rearrange("(k p) c -> p k c", p=128),
                    accum[:, :, (1 + 16) * B:(1 + 32) * B],
                )
                nc.gpsimd.collective_compute(
                    "AllGather", mybir.AluOpType.bypass, replica_groups=groups,
                    ins=[ag_in2.opt()], outs=[ag_out2[:].opt()],
                )

                # ---------- queue work that depends on AG2_k ----------
                pend_half1 = []
                if k + 1 < iters:
                    pend_half1 = make_ih_half(k + 1, 1, emb_next, ih_next, ag_out2)
                if k >= 1:
                    pend_half1 = make_dec_half(k - 1, 1, accum) + pend_half1
                if k + 1 >= 6 and (k + 1 - 6) % 4 == 0 and (k + 1 - 6) // 4 < 3:
                    post_queue.extend(make_post((k + 1 - 6) // 4))
                pend_post = post_queue[:12]
                post_queue = post_queue[12:]

                accum_prev = accum

            # ---------- tail: dec half1 of chunk 15 + post group 3 ----------
            for t in pend_half1:
                t()
            for t in post_queue:
                t()
            for t in make_post(3):
                t()

    nc.compile()
    return nc


# ---------------- host-side preparation ----------------

def _prep_inputs(inputs, nch=NCH):
    assert nch == NCH
    iters = ITERS
    perm = _gate_perm()

    trg = np.asarray(inputs["trg_inputs"]).astype(np.int64)
    trg_len = np.asarray(inputs["trg_len"]).astype(np.int64)
    enc = _f32(inputs["encoder_outputs"])
    h0 = _f32(inputs["h0"]).reshape(L, 2, B, H)
    c0 = _f32(inputs["c0"]).reshape(L, 2, B, H)
    embed = _f32(inputs["embed"])
    W_ih0 = _f32(inputs["W_ih0"])          # [2, 4H, E]
    W_ih1 = _f32(inputs["W_ih1"])[0]       # [2, 4H, 2H]
    W_hh = _f32(inputs["W_hh"])            # [L, 2, 4H, H]
    b_ih = _f32(inputs["b_ih"])            # [L, 2, 4H]
    b_hh = _f32(inputs["b_hh"])
    W_att = _f32(inputs["W_att"])          # [H, H]
    b_att = _f32(inputs["b_att"])          # [H]
    W_fc = _f32(inputs["W_fc"])            # [OUT, 2H]
    b_fc = _f32(inputs["b_fc"])            # [OUT]

    # embedding stream  [iters, E, COLS]; emb_stream[k,e,s*B+b] = X[b,32k+s,e]
    X = embed[trg[:, :T]]                             # [B, T, E]
    es = np.zeros((iters, E, COLS), np.float32)
    xt = X.transpose(2, 1, 0)                         # [E, T, B]
    es[:NCH] = (
        xt.reshape(E, NCH, CH, B).transpose(1, 0, 2, 3).reshape(NCH, E, COLS)
    )
    es = _bf(es)

    cells = [(0, 0), (0, 1), (1, 0), (1, 1)]          # (layer, dir)
    zeros_emb = _bf(np.zeros((E, 4 * H)))
    zeros_in = _bf(np.zeros((2 * H, 4 * H)))

    # masks in transposed layouts
    # maskT [B, 128, SC]: 0 where s < len else -1e30  (s = sk*128 + p)
    # -30 shift guards exp() against fp32 overflow (scores can reach ~90);
    # the shift cancels in the softmax normalization.
    s_idx = np.arange(S).reshape(SC, 128).T           # [128, SC]
    maskT = np.where(s_idx[None, :, :] < trg_len[:, None, None], -30.0, -1e30).astype(np.float32)
    # validT [B, 128, TG]: 1 where t < len else 0  (t = g*128 + p)
    t_idx = np.arange(T).reshape(TG, 128).T
    validT = (t_idx[None, :, :] < trg_len[:, None, None]).astype(np.float32)

    encT = enc.transpose(0, 2, 1)                     # [B, H, S]

    in_maps = []
    for c in range(N_CORES):
        cell = c % 4
        layer, d = cells[cell]
        if layer == 0:
            wemb = _bf(W_ih0[d][perm].T)              # [E, 4H]
            win = zeros_in
        else:
            wemb = zeros_emb
            win = _bf(W_ih1[d][perm].T)               # [2H, 4H]
        whh = _bf(W_hh[layer, d][perm].T)             # [H, 4H]
        bp = (b_ih[layer, d] + b_hh[layer, d])[perm]  # [4H]
        biasp = _f32(bp.reshape(MC, 128).T)           # [128, MC]
        hin = h0[layer, d]                            # [B, H]
        cin = c0[layer, d]
        h_init = _bf(hin.T.reshape(HC, 128, B).transpose(1, 0, 2))   # [128,HC,B]
        c_init = _f32(cin.T.reshape(HC, 128, B).transpose(1, 0, 2))
        a = 1.0 if layer == 0 else 0.0
        alpha = _f32(np.full((128, 1), a))
        beta = _f32(np.full((128, 1), 1.0 - a))
        gam = _f32(np.full((128, 1), 1.0 if c in (2, 3) else 0.0))

        bsl = [2 * c, 2 * c + 1]                      # post batches of this core

        m = dict(
            w_emb=wemb, w_in=win, w_hh=whh, biasp=biasp,
            h_init=h_init, c_init=c_init, alpha=alpha, beta=beta, gamma=gam,
            emb_stream=es,
            enc_lhsT=_bf(enc[bsl]),                   # [2, S, H]
            encT_rhs=_bf(encT[bsl]),                  # [2, H, S]
            w_attT=_bf(W_att.T),
            b_att_in=_f32(b_att.reshape(HC, 128).T),
            maskT_in=_f32(maskT[bsl]),                # [2,128,SC]
            valid_in=_f32(validT[bsl]),               # [2,128,TG]
            w_fcT=_bf(W_fc.T),                        # [2H, OUT]
            b_fc_row=_bf(b_fc[None, :]),
        )
        in_maps.append(m)
    return in_maps


_NC_CACHE = {}


def kernel(**inputs) -> np.ndarray:
    nch = int(os.environ.get("KERNEL_NCH", NCH))
    if nch not in _NC_CACHE:
        _NC_CACHE[nch] = build_nc(nch)
    nc = _NC_CACHE[nch]
    in_maps = _prep_inputs(inputs, nch)
    r = run_bass_kernel_spmd(nc, in_maps, list(range(N_CORES)))
    outs = [np.asarray(r.results[c]["out"], np.float32) for c in range(N_CORES)]
    return np.concatenate(outs, axis=0)
